# revision 34
# baseline (speedup 1.0000x reference)
"""Trainium2 Bass kernel for nn_DCTModel: bilinear x8 upsample + RGB->YCbCr +
8x8 block DCT + channel selection, fused into two dense matmuls per plane.

Math: the whole reference pipeline is linear in x (all affine offsets only
shift the DC coefficient, which is excluded from the output), so

    out[b, r, (u,i), (v,j)] = (Th @ Xhat[b,r] @ Tw^T)[(u,i), (v,j)]

with Xhat[b,r] = sum_c 127.5*RGB2YCBCR[r,c] * x[b,c]  (112x112),
Th = C @ Ah (DCT-harmonics x bilinear-upsample, [8*112, 112]) with the
orthonormal alpha(u)/2 scale folded in; Tw identical. 54 of the 64 (u,v)
DCT channels are kept.

On-chip per (b, r) plane (the mix is normalized by 1/M2[r,1] so it is only
2 DVE ops; the scale is re-applied for free in the a1t scale-on-copy; ThT
and TwT are the same matrix so only one 200KB constant is loaded):
  mix (DVE)            Xh' = (a0/a1) x0 + x1 + (a2/a1) x2  -> fp16 [112,112]
  matmul 1 (PE, fp16)  A1t[w,(u,i)] = a1 * (Xh'^T @ ThT)   -> PSUM -> fp16
  matmul 2 (PE, fp16)  Yu[i,(v,j)]  = A1t[:,u-slice]^T @ TwT -> PSUM f32
  copies (DVE+ACT)     PSUM (bank-packed u-groups) -> staging tile (fp32)
  DMA (sync+ACT HWDGE) staging -> out[b, :, ch0:ch1, :] per u-group

Head optimizations: 6 input DMAs all in ring position 1-2 (per-ring sem
visibility ~9.4/~10.6us after the ~7us Tile preamble), plane-0 emission
interleaves matmul1 halves with the matmul2 groups they unlock (engines
execute their streams in order), plane-0 copies are explicitly pinned
alternating DVE/ACT, and 7 dummy matmuls warm the PE HAM clock-gate to
2.4 GHz during the otherwise-idle input-load window.

The DRAM output tensor is laid out [b, i, c, j] (spatial-row major) rather
than [b, c, i, j]: the SBUF staging tile [i-partition, (c j)] then maps to
DRAM with an (c j)-contiguous run per partition, so every DMA descriptor is
nsel*448 bytes (0.9-3.5 KB) instead of 448 B.  448 B descriptors sit below
the SDMA 512 B line-rate minimum and measured ~15 GB/s/engine (240 GB/s
aggregate); the big-descriptor form reaches the ~300+ GB/s HBM-per-core
practical limit.  The host unshards with a numpy transpose back to
[b, c, i, j].

fp16 keeps |values| < ~1e3 (well in range); measured end-to-end rel err
~4e-4 vs the fp32 reference.

Sharding: pure data parallel, batch 16 -> 2 per core across 8 cores.
"""

import numpy as np

L = 112
SIZE = 8
BS_PER_CORE = 2
N_CORES = 8
SUB_CHANNELS = {0, 1, 2, 3, 4, 5, 8, 9, 16, 24}

RGB2YCBCR = np.asarray(
    [[0.299, 0.587, 0.114],
     [-0.168736, -0.331264, 0.5],
     [0.5, -0.418688, -0.081312]], np.float32)

# per-u: first selected v (selected v's are the contiguous range [V_LO[u], 8))
V_LO = []
M_START = []
_m = 0
for _u in range(SIZE):
    _sel = [_v for _v in range(SIZE) if _u * SIZE + _v not in SUB_CHANNELS]
    assert _sel == list(range(_sel[0], SIZE))
    V_LO.append(_sel[0])
    M_START.append(_m)
    _m += len(_sel)
assert _m == 54


def _build_consts():
    """ThT[h', u*112+i] = alpha(u)/2 * sum_x h[x,u] * Ah[8i+x, h']  (fp16).

    ThT == TwT (H and W are symmetric); a single [112, 896] fp16 matrix
    serves as the moving operand of both matmuls.
    """
    Lo = L * SIZE
    src = np.arange(Lo) * (L - 1) / (Lo - 1)
    i0 = np.minimum(np.floor(src).astype(np.int64), L - 2)
    w = (src - i0).astype(np.float32)
    A = np.zeros((Lo, L), np.float32)
    A[np.arange(Lo), i0] = 1.0 - w
    A[np.arange(Lo), i0 + 1] = w

    xg = np.arange(SIZE) + 0.5
    ug = np.arange(SIZE)
    h = np.cos(np.outer(xg, ug) * np.pi / SIZE).astype(np.float32)
    alpha = np.ones(SIZE, np.float32)
    alpha[0] = 1.0 / np.sqrt(2.0)

    Ab = A.reshape(L, SIZE, L)  # [i, x, h']
    Th = np.einsum('xu,ixh->uih', h, Ab).astype(np.float32)
    Th = Th * (alpha / 2.0)[:, None, None]
    ThT = np.ascontiguousarray(Th.transpose(2, 0, 1).reshape(L, SIZE * L))
    # The same matrix serves as matmul1's rhs (ThT) and matmul2's rhs (TwT);
    # only one copy is loaded.
    return ThT.astype(np.float16)


_CACHE = {}


def _build_program():
    import concourse.bacc as bacc
    import concourse.mybir as mybir
    import concourse.tile as tile

    f32 = mybir.dt.float32
    f16 = mybir.dt.float16
    mult = mybir.AluOpType.mult
    add = mybir.AluOpType.add

    M2 = (127.5 * RGB2YCBCR).astype(np.float32)

    nc = bacc.Bacc(
        "TRN2",
        target_bir_lowering=False,
        debug=False,
        enable_asserts=False,
        num_devices=N_CORES,
        # the kernel never reads partition_id; dropping it removes the
        # per-engine preamble TENSOR_LOADs (~1.1us each) that fetch it.
        enable_partition_id=False,
    )
    x_d = nc.dram_tensor("x", [BS_PER_CORE, 3, L, L], f32, kind="ExternalInput").ap()
    twt_d = nc.dram_tensor("twt", [L, SIZE * L], f16, kind="ExternalInput").ap()
    # [b, i, c, j]: row-major in the spatial row so the per-u DMA gets
    # (c j)-contiguous DRAM runs per partition (host transposes back).
    out_d = nc.dram_tensor(
        "out", [BS_PER_CORE, L, 162, L], f32, kind="ExternalOutput"
    ).ap()

    with tile.TileContext(nc) as tc:
        with tc.tile_pool(name="consts", bufs=1) as cpool, \
             tc.tile_pool(name="xin", bufs=2) as xpool, \
             tc.tile_pool(name="mix", bufs=3) as mpool, \
             tc.tile_pool(name="work", bufs=3) as wpool, \
             tc.tile_pool(name="outb", bufs=16) as opool, \
             tc.tile_pool(name="ps", bufs=2, space="PSUM") as ppool, \
             tc.tile_pool(name="ps2", bufs=3, space="PSUM") as ppool2:
            # b=0 heads the critical path: load its channels as three
            # parallel DMAs (c0/c1 feed the first mix op) split across both
            # HWDGE rings so the first mix starts as early as possible.
            xbs = []
            for b in range(BS_PER_CORE):
                xb = xpool.tile([L, 3, L], f32, name=f"xb{b}", tag="xb")
                xbs.append(xb)
            twt = cpool.tile([L, SIZE * L], f16, name="twt_sb")
            # 6 input DMAs, every one in ring position 1 or 2 (per-ring sem
            # visibility: pos1 ~9.4, pos2 ~10.6).  Input loading runs at
            # only ~180 GB/s aggregate (448-2048B descriptors), so total
            # input bytes (501KB) set the input-phase length.
            nc.sync.dma_start(xbs[0][:, 0, :], x_d[0, 0])
            nc.scalar.dma_start(xbs[0][:, 1, :], x_d[0, 1])
            nc.gpsimd.dma_start(xbs[0][:, 2, :], x_d[0, 2])
            nc.sync.dma_start(twt[:, 0:448], twt_d[:, 0:448])
            nc.scalar.dma_start(twt[:, 448:896], twt_d[:, 448:896])
            nc.gpsimd.dma_start(xbs[1][:], x_d[1].transpose([1, 0, 2]))

            # Balance PSUM->SBUF copies between DVE and ACT by measured ns
            # (cayman-errata: DVE (265+FD)/0.96, ACT (475+FD)/1.2 for f32
            # PSUM reads).  DVE also carries the per-plane mix; ACT also
            # issues half the output DMAs (both charged to the tallies).
            eng_ns = {"v": 0.0, "s": 0.0}

            def psum_copy(dst, src, ncols, scale=None, eng=None):
                v_cost = (265.0 + ncols) / 0.96
                s_cost = (475.0 + ncols) / 1.2
                if eng is None:
                    eng = ("v" if eng_ns["v"] + v_cost <= eng_ns["s"] + s_cost
                           else "s")
                if eng == "v":
                    if scale is None:
                        nc.vector.tensor_copy(dst, src)
                    else:
                        nc.vector.tensor_scalar_mul(dst, src, scale)
                    eng_ns["v"] += v_cost
                else:
                    if scale is None:
                        nc.scalar.copy(dst, src)
                    else:
                        nc.scalar.mul(dst, src, scale)
                    eng_ns["s"] += s_cost

            n_dma = [0]

            def emit_mix(b, r):
                """RGB->normalized-Y'CbCr' plane mix on DVE; returns xh."""
                xb = xbs[b]
                tmp = mpool.tile([L, L], f32, name=f"tmp{b}{r}", tag="tmp")
                xh = mpool.tile([L, L], f16, name=f"xh{b}{r}", tag="xh")
                # Xh' = (a0/a1) x0 + x1 + (a2/a1) x2; a1 re-applied by the
                # a1t scale-on-copy.
                nc.vector.scalar_tensor_tensor(
                    tmp[:], xb[:, 0, :], float(M2[r, 0] / M2[r, 1]),
                    xb[:, 1, :], mult, add)
                nc.vector.scalar_tensor_tensor(
                    xh[:], xb[:, 2, :], float(M2[r, 2] / M2[r, 1]),
                    tmp[:], mult, add)
                eng_ns["v"] += 2 * 330.0
                return xh

            def emit_mm1_slice(b, r, a1t, xh, lo, hi, eng=None):
                """matmul1 cols [lo:hi): A1t[w, (u,i)] = a1*(Xh'^T @ ThT).

                ThT and TwT are the same matrix; matmul1 reuses the twt
                tile as its moving operand."""
                w = hi - lo
                ps = ppool.tile([L, 512], f32, name=f"psA{b}{r}{lo}",
                                tag="ps")
                nc.tensor.matmul(
                    ps[0:L, 0:w],
                    lhsT=xh[:],
                    rhs=twt[:, lo:hi],
                    start=True, stop=True)
                # the mix's 1/M2[r,1] normalization is undone here for
                # free (scale-on-copy).
                psum_copy(a1t[:, lo:hi], ps[0:L, 0:w], w,
                          scale=float(M2[r, 1]), eng=eng)

            def emit_stage1(k, b, r):
                """mix + matmul1 for plane k; returns the a1t tile."""
                xh = emit_mix(b, r)
                a1t = wpool.tile([L, SIZE * L], f16, name=f"a1t{b}{r}",
                                 tag="a1t")
                for half in range(2):
                    emit_mm1_slice(b, r, a1t, xh, 448 * half, 448 * (half + 1))
                return a1t

            # matmul2 u-groups: each group's chunks are packed into one
            # 2-bank PSUM tile at offsets chosen so (a) every matmul output
            # stays inside a single 2KB bank and (b) the used columns are
            # one contiguous run -> ONE PSUM->SBUF copy per group instead
            # of two (the per-op overhead is ~265-475 engine cycles).
            # (us, twt_lo, twt_hi, psum_off) per chunk; m channel range.
            GROUPS = []
            GROUPS.append(([(0, 672, 896, 64), (1, 224, 448, 288),
                            (1, 448, 896, 512)], 0, 8, 64))
            GROUPS.append(([(2, 112, 448, 176), (2, 448, 896, 512)],
                           8, 15, 176))
            GROUPS.append(([(3, 112, 448, 176), (3, 448, 896, 512)],
                           15, 22, 176))
            for _u in range(4, 8):
                GROUPS.append(([(_u, 0, 448, 64), (_u, 448, 896, 512)],
                               22 + (_u - 4) * 8, 30 + (_u - 4) * 8, 64))
            # plane-0 variant: u0 and u1 split so the first (tiny) DMA
            # fires as early as possible.
            GROUPS0 = [([(0, 672, 896, 0)], 0, 2, 0),
                       ([(1, 224, 448, 288), (1, 448, 896, 512)], 2, 8, 288)]
            GROUPS0 += GROUPS[1:]

            def emit_group(b, r, a1t, gi, group, eng=None):
                """matmul2 + staging copy + output DMA for one u-group."""
                chunks, mlo, mhi, off = group
                ncols = (mhi - mlo) * L
                pp = ppool2.tile([128, 1024], f32, name=f"pp{b}{r}{gi}",
                                 tag="pp")
                for us, tlo, thi, poff in chunks:
                    nc.tensor.matmul(
                        pp[0:L, poff:poff + (thi - tlo)],
                        lhsT=a1t[:, us * L:(us + 1) * L],
                        rhs=twt[:, tlo:thi],
                        start=True, stop=True)
                ust = opool.tile([L, ncols], f32,
                                 name=f"ust{b}{r}{gi}", tag="ust")
                psum_copy(ust[:], pp[0:L, off:off + ncols], ncols, eng=eng)

                # dst [i, c, j]: per partition i the (c, j) run is
                # contiguous in DRAM -> one (mhi-mlo)*448B descriptor
                # per partition.  1:1 over the two HWDGE rings (ACT's
                # issue time is charged to its copy balance).
                c0 = r * 54 + mlo
                if n_dma[0] % 2 == 1:
                    nc.scalar.dma_start(out_d[b, :, c0:c0 + (mhi - mlo)],
                                        ust[:])
                    eng_ns["s"] += 700.0
                else:
                    nc.sync.dma_start(out_d[b, :, c0:c0 + (mhi - mlo)],
                                      ust[:])
                n_dma[0] += 1

            def emit_stage2(b, r, a1t):
                for gi, g in enumerate(GROUPS):
                    emit_group(b, r, a1t, gi, g)

            # Plane 0 heads the critical path.  Engines execute their
            # instruction streams IN ORDER, so matmul1 slices are emitted
            # interleaved with exactly the matmul2 groups they unlock: a
            # slice gated on a late tht quarter-load must not precede (in
            # PE program order) groups whose inputs are already resident.
            # HAM warm-up: the PE powers on clock-gated at 1.2 GHz and
            # needs ~3.4us of activity to reach 2.4 GHz.  The PE is idle
            # during the input-load window anyway, so burn it with dummy
            # matmuls on an uninitialized scratch tile (results land in a
            # PSUM bank that every real matmul later overwrites with
            # start=True); the real plane-0 stream then runs at full clock.
            dmy = cpool.tile([L, 512], f16, name="ham_warm")
            nc.vector.memset(dmy[:], 1.0)
            for hw_i in range(7):
                psd = ppool2.tile([128, 1024], f32, name=f"hw{hw_i}",
                                  tag="pp")
                nc.tensor.matmul(
                    psd[0:L, 0:512],
                    lhsT=dmy[:, 0:112],
                    rhs=dmy[:],
                    start=True, stop=True)

            # Copies in the plane-0 block are explicitly pinned, strictly
            # alternating DVE/ACT in gate order: the greedy balancer would
            # otherwise queue a critical copy behind hoisted later-plane
            # mixes on DVE while ACT sits idle (seen in profile).
            b0, r0 = 0, 0
            xh0 = emit_mix(b0, r0)
            a1t0 = wpool.tile([L, SIZE * L], f16, name="a1t00", tag="a1t")
            # a1t[0:224] is the stationary for BOTH u0 and u1: one 510ns
            # copy unlocks the first 8 output channels (0.4MB), filling the
            # DMA queue through the 12.5-14us bridge before backlog builds.
            emit_mm1_slice(b0, r0, a1t0, xh0, 0, 224, eng="v")    # u0+u1 lhsT
            emit_mm1_slice(b0, r0, a1t0, xh0, 224, 448, eng="s")  # u2+u3 lhsT
            emit_group(b0, r0, a1t0, 0, GROUPS0[0], eng="v")     # u0
            emit_group(b0, r0, a1t0, 1, GROUPS0[1], eng="v")     # u1
            emit_group(b0, r0, a1t0, 2, GROUPS0[2], eng="s")     # u2
            emit_group(b0, r0, a1t0, 3, GROUPS0[3], eng="v")     # u3
            emit_mm1_slice(b0, r0, a1t0, xh0, 448, 896, eng="s")  # u4-u7 lhsT
            for gi in range(4, 8):
                emit_group(b0, r0, a1t0, gi, GROUPS0[gi],
                           eng=("v" if gi % 2 == 0 else "s"))    # u4-u7

            # Remaining planes, software-pipelined: plane k+1's mix/matmul1
            # is emitted before plane k's matmul2 stream, so the PE never
            # idles across the a1t copy boundary between planes.
            planes = [(b, r) for b in range(BS_PER_CORE) for r in range(3)][1:]
            prev = None
            for k, (b, r) in enumerate(planes):
                a1t = emit_stage1(k, b, r)
                if prev is not None:
                    emit_stage2(prev[0], prev[1], prev[2])
                prev = (b, r, a1t)
            emit_stage2(prev[0], prev[1], prev[2])

    nc.compile()
    return nc


def kernel(x: np.ndarray) -> np.ndarray:
    from concourse import bass_utils

    x = np.ascontiguousarray(np.asarray(x, np.float32))
    assert x.shape == (BS_PER_CORE * N_CORES, 3, L, L)

    if "nc" not in _CACHE:
        _CACHE["nc"] = _build_program()
        _CACHE["consts"] = _build_consts()
    nc = _CACHE["nc"]
    TwT = _CACHE["consts"]

    in_maps = [
        {"x": x[c * BS_PER_CORE:(c + 1) * BS_PER_CORE], "twt": TwT}
        for c in range(N_CORES)
    ]
    res = bass_utils.run_bass_kernel_spmd(nc, in_maps, core_ids=list(range(N_CORES)))
    # [16, 112, 162, 112] (b, i, c, j) -> [16, 162, 112, 112]
    out = np.concatenate([res.results[c]["out"] for c in range(N_CORES)], axis=0)
    return np.ascontiguousarray(out.transpose(0, 2, 1, 3))


# revision 35
# speedup vs baseline: 1.0199x; 1.0199x over previous
"""Trainium2 Bass kernel for nn_DCTModel: bilinear x8 upsample + RGB->YCbCr +
8x8 block DCT + channel selection, fused into two dense matmuls per plane.

Math: the whole reference pipeline is linear in x (all affine offsets only
shift the DC coefficient, which is excluded from the output), so

    out[b, r, (u,i), (v,j)] = (Th @ Xhat[b,r] @ Tw^T)[(u,i), (v,j)]

with Xhat[b,r] = sum_c 127.5*RGB2YCBCR[r,c] * x[b,c]  (112x112),
Th = C @ Ah (DCT-harmonics x bilinear-upsample, [8*112, 112]) with the
orthonormal alpha(u)/2 scale folded in; Tw identical. 54 of the 64 (u,v)
DCT channels are kept.

On-chip per (b, r) plane (the mix is normalized by 1/M2[r,1] so it is only
2 DVE ops; the scale is re-applied for free in the a1t scale-on-copy; ThT
and TwT are the same matrix so only one 200KB constant is loaded):
  mix (DVE)            Xh' = (a0/a1) x0 + x1 + (a2/a1) x2  -> fp16 [112,112]
  matmul 1 (PE, fp16)  A1t[w,(u,i)] = a1 * (Xh'^T @ ThT)   -> PSUM -> fp16
  matmul 2 (PE, fp16)  Yu[i,(v,j)]  = A1t[:,u-slice]^T @ TwT -> PSUM f32
  copies (DVE+ACT)     PSUM (bank-packed u-groups) -> staging tile (fp32)
  DMA (sync+ACT HWDGE) staging -> out[b, :, ch0:ch1, :] per u-group

Head optimizations: 6 input DMAs all in ring position 1-2 (per-ring sem
visibility ~9.4/~10.6us after the ~7us Tile preamble), plane-0 emission
interleaves matmul1 halves with the matmul2 groups they unlock (engines
execute their streams in order), plane-0 copies are explicitly pinned
alternating DVE/ACT, and 7 dummy matmuls warm the PE HAM clock-gate to
2.4 GHz during the otherwise-idle input-load window.

The DRAM output tensor is laid out [b, i, c, j] (spatial-row major) rather
than [b, c, i, j]: the SBUF staging tile [i-partition, (c j)] then maps to
DRAM with an (c j)-contiguous run per partition, so every DMA descriptor is
nsel*448 bytes (0.9-3.5 KB) instead of 448 B.  448 B descriptors sit below
the SDMA 512 B line-rate minimum and measured ~15 GB/s/engine (240 GB/s
aggregate); the big-descriptor form reaches the ~300+ GB/s HBM-per-core
practical limit.  The host unshards with a numpy transpose back to
[b, c, i, j].

fp16 keeps |values| < ~1e3 (well in range); measured end-to-end rel err
~4e-4 vs the fp32 reference.

Sharding: pure data parallel, batch 16 -> 2 per core across 8 cores.
"""

import numpy as np

L = 112
SIZE = 8
BS_PER_CORE = 2
N_CORES = 8
SUB_CHANNELS = {0, 1, 2, 3, 4, 5, 8, 9, 16, 24}

RGB2YCBCR = np.asarray(
    [[0.299, 0.587, 0.114],
     [-0.168736, -0.331264, 0.5],
     [0.5, -0.418688, -0.081312]], np.float32)

# per-u: first selected v (selected v's are the contiguous range [V_LO[u], 8))
V_LO = []
M_START = []
_m = 0
for _u in range(SIZE):
    _sel = [_v for _v in range(SIZE) if _u * SIZE + _v not in SUB_CHANNELS]
    assert _sel == list(range(_sel[0], SIZE))
    V_LO.append(_sel[0])
    M_START.append(_m)
    _m += len(_sel)
assert _m == 54


def _build_consts():
    """ThT[h', u*112+i] = alpha(u)/2 * sum_x h[x,u] * Ah[8i+x, h']  (fp16).

    ThT == TwT (H and W are symmetric); a single [112, 896] fp16 matrix
    serves as the moving operand of both matmuls.
    """
    Lo = L * SIZE
    src = np.arange(Lo) * (L - 1) / (Lo - 1)
    i0 = np.minimum(np.floor(src).astype(np.int64), L - 2)
    w = (src - i0).astype(np.float32)
    A = np.zeros((Lo, L), np.float32)
    A[np.arange(Lo), i0] = 1.0 - w
    A[np.arange(Lo), i0 + 1] = w

    xg = np.arange(SIZE) + 0.5
    ug = np.arange(SIZE)
    h = np.cos(np.outer(xg, ug) * np.pi / SIZE).astype(np.float32)
    alpha = np.ones(SIZE, np.float32)
    alpha[0] = 1.0 / np.sqrt(2.0)

    Ab = A.reshape(L, SIZE, L)  # [i, x, h']
    Th = np.einsum('xu,ixh->uih', h, Ab).astype(np.float32)
    Th = Th * (alpha / 2.0)[:, None, None]
    ThT = np.ascontiguousarray(Th.transpose(2, 0, 1).reshape(L, SIZE * L))
    # The same matrix serves as matmul1's rhs (ThT) and matmul2's rhs (TwT);
    # only one copy is loaded.
    return ThT.astype(np.float16)


_CACHE = {}


def _build_program():
    import concourse.bacc as bacc
    import concourse.mybir as mybir
    import concourse.tile as tile

    f32 = mybir.dt.float32
    f16 = mybir.dt.float16
    mult = mybir.AluOpType.mult
    add = mybir.AluOpType.add

    M2 = (127.5 * RGB2YCBCR).astype(np.float32)

    nc = bacc.Bacc(
        "TRN2",
        target_bir_lowering=False,
        debug=False,
        enable_asserts=False,
        num_devices=N_CORES,
        # the kernel never reads partition_id; dropping it removes the
        # per-engine preamble TENSOR_LOADs (~1.1us each) that fetch it.
        enable_partition_id=False,
    )
    x_d = nc.dram_tensor("x", [BS_PER_CORE, 3, L, L], f32, kind="ExternalInput").ap()
    twt_d = nc.dram_tensor("twt", [L, SIZE * L], f16, kind="ExternalInput").ap()
    # [b, i, c, j]: row-major in the spatial row so the per-u DMA gets
    # (c j)-contiguous DRAM runs per partition (host transposes back).
    out_d = nc.dram_tensor(
        "out", [BS_PER_CORE, L, 162, L], f32, kind="ExternalOutput"
    ).ap()

    with tile.TileContext(nc) as tc:
        with tc.tile_pool(name="consts", bufs=1) as cpool, \
             tc.tile_pool(name="xin", bufs=2) as xpool, \
             tc.tile_pool(name="mix", bufs=3) as mpool, \
             tc.tile_pool(name="work", bufs=3) as wpool, \
             tc.tile_pool(name="outb", bufs=16) as opool, \
             tc.tile_pool(name="ps", bufs=2, space="PSUM") as ppool, \
             tc.tile_pool(name="ps2", bufs=3, space="PSUM") as ppool2:
            # b=0 heads the critical path: load its channels as three
            # parallel DMAs (c0/c1 feed the first mix op) split across both
            # HWDGE rings so the first mix starts as early as possible.
            xbs = []
            for b in range(BS_PER_CORE):
                xb = xpool.tile([L, 3, L], f32, name=f"xb{b}", tag="xb")
                xbs.append(xb)
            twt = cpool.tile([L, SIZE * L], f16, name="twt_sb")
            # 6 input DMAs, every one in ring position 1 or 2 (per-ring sem
            # visibility: pos1 ~9.4, pos2 ~10.6).  Input loading runs at
            # only ~180 GB/s aggregate (448-2048B descriptors), so total
            # input bytes (501KB) set the input-phase length.
            nc.sync.dma_start(xbs[0][:, 0, :], x_d[0, 0])
            nc.scalar.dma_start(xbs[0][:, 1, :], x_d[0, 1])
            nc.gpsimd.dma_start(xbs[0][:, 2, :], x_d[0, 2])
            nc.sync.dma_start(twt[:, 0:448], twt_d[:, 0:448])
            nc.scalar.dma_start(twt[:, 448:896], twt_d[:, 448:896])
            nc.gpsimd.dma_start(xbs[1][:], x_d[1].transpose([1, 0, 2]))

            # Balance PSUM->SBUF copies between DVE and ACT by measured ns
            # (cayman-errata: DVE (265+FD)/0.96, ACT (475+FD)/1.2 for f32
            # PSUM reads).  DVE also carries the per-plane mix; ACT also
            # issues half the output DMAs (both charged to the tallies).
            eng_ns = {"v": 0.0, "s": 0.0}

            def psum_copy(dst, src, ncols, scale=None, eng=None):
                v_cost = (265.0 + ncols) / 0.96
                s_cost = (475.0 + ncols) / 1.2
                if eng is None:
                    eng = ("v" if eng_ns["v"] + v_cost <= eng_ns["s"] + s_cost
                           else "s")
                if eng == "v":
                    if scale is None:
                        nc.vector.tensor_copy(dst, src)
                    else:
                        nc.vector.tensor_scalar_mul(dst, src, scale)
                    eng_ns["v"] += v_cost
                else:
                    if scale is None:
                        nc.scalar.copy(dst, src)
                    else:
                        nc.scalar.mul(dst, src, scale)
                    eng_ns["s"] += s_cost

            n_dma = [0]

            def emit_mix(b, r):
                """RGB->normalized-Y'CbCr' plane mix on DVE; returns xh."""
                xb = xbs[b]
                tmp = mpool.tile([L, L], f32, name=f"tmp{b}{r}", tag="tmp")
                xh = mpool.tile([L, L], f16, name=f"xh{b}{r}", tag="xh")
                # Xh' = (a0/a1) x0 + x1 + (a2/a1) x2; a1 re-applied by the
                # a1t scale-on-copy.
                nc.vector.scalar_tensor_tensor(
                    tmp[:], xb[:, 0, :], float(M2[r, 0] / M2[r, 1]),
                    xb[:, 1, :], mult, add)
                nc.vector.scalar_tensor_tensor(
                    xh[:], xb[:, 2, :], float(M2[r, 2] / M2[r, 1]),
                    tmp[:], mult, add)
                eng_ns["v"] += 2 * 330.0
                return xh

            def emit_mm1_slice(b, r, a1t, xh, lo, hi, eng=None):
                """matmul1 cols [lo:hi): A1t[w, (u,i)] = a1*(Xh'^T @ ThT).

                ThT and TwT are the same matrix; matmul1 reuses the twt
                tile as its moving operand."""
                w = hi - lo
                ps = ppool.tile([L, 512], f32, name=f"psA{b}{r}{lo}",
                                tag="ps")
                nc.tensor.matmul(
                    ps[0:L, 0:w],
                    lhsT=xh[:],
                    rhs=twt[:, lo:hi],
                    start=True, stop=True)
                # the mix's 1/M2[r,1] normalization is undone here for
                # free (scale-on-copy).
                psum_copy(a1t[:, lo:hi], ps[0:L, 0:w], w,
                          scale=float(M2[r, 1]), eng=eng)

            def emit_stage1(k, b, r):
                """mix + matmul1 for plane k; returns the a1t tile."""
                xh = emit_mix(b, r)
                a1t = wpool.tile([L, SIZE * L], f16, name=f"a1t{b}{r}",
                                 tag="a1t")
                for half in range(2):
                    emit_mm1_slice(b, r, a1t, xh, 448 * half, 448 * (half + 1))
                return a1t

            # matmul2 u-groups: each group's chunks are packed into one
            # 2-bank PSUM tile at offsets chosen so (a) every matmul output
            # stays inside a single 2KB bank and (b) the used columns are
            # one contiguous run -> ONE PSUM->SBUF copy per group instead
            # of two (the per-op overhead is ~265-475 engine cycles).
            # (us, twt_lo, twt_hi, psum_off) per chunk; m channel range.
            GROUPS = []
            GROUPS.append(([(0, 672, 896, 64), (1, 224, 448, 288),
                            (1, 448, 896, 512)], 0, 8, 64))
            GROUPS.append(([(2, 112, 448, 176), (2, 448, 896, 512)],
                           8, 15, 176))
            GROUPS.append(([(3, 112, 448, 176), (3, 448, 896, 512)],
                           15, 22, 176))
            for _u in range(4, 8):
                GROUPS.append(([(_u, 0, 448, 64), (_u, 448, 896, 512)],
                               22 + (_u - 4) * 8, 30 + (_u - 4) * 8, 64))
            # plane-0 variant: u0 and u1 split so the first (tiny) DMA
            # fires as early as possible.
            GROUPS0 = [([(0, 672, 896, 0)], 0, 2, 0),
                       ([(1, 224, 448, 288), (1, 448, 896, 512)], 2, 8, 288)]
            GROUPS0 += GROUPS[1:]

            def emit_group(b, r, a1t, gi, group, eng=None):
                """matmul2 + staging copy + output DMA for one u-group."""
                chunks, mlo, mhi, off = group
                ncols = (mhi - mlo) * L
                pp = ppool2.tile([128, 1024], f32, name=f"pp{b}{r}{gi}",
                                 tag="pp")
                for us, tlo, thi, poff in chunks:
                    nc.tensor.matmul(
                        pp[0:L, poff:poff + (thi - tlo)],
                        lhsT=a1t[:, us * L:(us + 1) * L],
                        rhs=twt[:, tlo:thi],
                        start=True, stop=True)
                ust = opool.tile([L, ncols], f32,
                                 name=f"ust{b}{r}{gi}", tag="ust")
                psum_copy(ust[:], pp[0:L, off:off + ncols], ncols, eng=eng)

                # dst [i, c, j]: per partition i the (c, j) run is
                # contiguous in DRAM -> one (mhi-mlo)*448B descriptor
                # per partition.  1:1 over the two HWDGE rings (ACT's
                # issue time is charged to its copy balance).
                c0 = r * 54 + mlo
                if n_dma[0] % 2 == 1:
                    nc.scalar.dma_start(out_d[b, :, c0:c0 + (mhi - mlo)],
                                        ust[:])
                    eng_ns["s"] += 700.0
                else:
                    nc.sync.dma_start(out_d[b, :, c0:c0 + (mhi - mlo)],
                                      ust[:])
                n_dma[0] += 1

            def emit_stage2(b, r, a1t):
                for gi, g in enumerate(GROUPS):
                    emit_group(b, r, a1t, gi, g)

            # Plane 0 heads the critical path.  Engines execute their
            # instruction streams IN ORDER, so matmul1 slices are emitted
            # interleaved with exactly the matmul2 groups they unlock: a
            # slice gated on a late tht quarter-load must not precede (in
            # PE program order) groups whose inputs are already resident.
            # HAM warm-up: the PE powers on clock-gated at 1.2 GHz and
            # needs ~3.4us of activity to reach 2.4 GHz.  The PE is idle
            # during the input-load window anyway, so burn it with dummy
            # matmuls on an uninitialized scratch tile (results land in a
            # PSUM bank that every real matmul later overwrites with
            # start=True); the real plane-0 stream then runs at full clock.
            dmy = cpool.tile([L, 512], f16, name="ham_warm")
            nc.vector.memset(dmy[:], 1.0)
            for hw_i in range(7):
                psd = ppool2.tile([128, 1024], f32, name=f"hw{hw_i}",
                                  tag="pp")
                nc.tensor.matmul(
                    psd[0:L, 0:512],
                    lhsT=dmy[:, 0:112],
                    rhs=dmy[:],
                    start=True, stop=True)

            # Copies in the plane-0 block are explicitly pinned, strictly
            # alternating DVE/ACT in gate order: the greedy balancer would
            # otherwise queue a critical copy behind hoisted later-plane
            # mixes on DVE while ACT sits idle (seen in profile).
            b0, r0 = 0, 0
            xh0 = emit_mix(b0, r0)
            a1t0 = wpool.tile([L, SIZE * L], f16, name="a1t00", tag="a1t")
            emit_mm1_slice(b0, r0, a1t0, xh0, 0, 448, eng="v")   # u0-u3 lhsT
            emit_group(b0, r0, a1t0, 0, GROUPS0[0], eng="s")     # u0
            emit_group(b0, r0, a1t0, 1, GROUPS0[1], eng="v")     # u1
            emit_group(b0, r0, a1t0, 2, GROUPS0[2], eng="s")     # u2
            emit_group(b0, r0, a1t0, 3, GROUPS0[3], eng="v")     # u3
            emit_mm1_slice(b0, r0, a1t0, xh0, 448, 896, eng="s")  # u4-u7 lhsT
            for gi in range(4, 8):
                emit_group(b0, r0, a1t0, gi, GROUPS0[gi],
                           eng=("v" if gi % 2 == 0 else "s"))    # u4-u7

            # Remaining planes, software-pipelined: plane k+1's mix/matmul1
            # is emitted before plane k's matmul2 stream, so the PE never
            # idles across the a1t copy boundary between planes.
            planes = [(b, r) for b in range(BS_PER_CORE) for r in range(3)][1:]
            prev = None
            for k, (b, r) in enumerate(planes):
                a1t = emit_stage1(k, b, r)
                if prev is not None:
                    emit_stage2(prev[0], prev[1], prev[2])
                prev = (b, r, a1t)
            emit_stage2(prev[0], prev[1], prev[2])

    nc.compile()
    return nc


def kernel(x: np.ndarray) -> np.ndarray:
    from concourse import bass_utils

    x = np.ascontiguousarray(np.asarray(x, np.float32))
    assert x.shape == (BS_PER_CORE * N_CORES, 3, L, L)

    if "nc" not in _CACHE:
        _CACHE["nc"] = _build_program()
        _CACHE["consts"] = _build_consts()
    nc = _CACHE["nc"]
    TwT = _CACHE["consts"]

    in_maps = [
        {"x": x[c * BS_PER_CORE:(c + 1) * BS_PER_CORE], "twt": TwT}
        for c in range(N_CORES)
    ]
    res = bass_utils.run_bass_kernel_spmd(nc, in_maps, core_ids=list(range(N_CORES)))
    # [16, 112, 162, 112] (b, i, c, j) -> [16, 162, 112, 112]
    out = np.concatenate([res.results[c]["out"] for c in range(N_CORES)], axis=0)
    return np.ascontiguousarray(out.transpose(0, 2, 1, 3))


# revision 36
# speedup vs baseline: 1.0241x; 1.0042x over previous
"""Trainium2 Bass kernel for nn_DCTModel: bilinear x8 upsample + RGB->YCbCr +
8x8 block DCT + channel selection, fused into two dense matmuls per plane.

Math: the whole reference pipeline is linear in x (all affine offsets only
shift the DC coefficient, which is excluded from the output), so

    out[b, r, (u,i), (v,j)] = (Th @ Xhat[b,r] @ Tw^T)[(u,i), (v,j)]

with Xhat[b,r] = sum_c 127.5*RGB2YCBCR[r,c] * x[b,c]  (112x112),
Th = C @ Ah (DCT-harmonics x bilinear-upsample, [8*112, 112]) with the
orthonormal alpha(u)/2 scale folded in; Tw identical. 54 of the 64 (u,v)
DCT channels are kept.

On-chip per (b, r) plane (the mix is normalized by 1/M2[r,1] so it is only
2 DVE ops; the scale is re-applied for free in the a1t scale-on-copy; ThT
and TwT are the same matrix so only one 200KB constant is loaded):
  mix (DVE)            Xh' = (a0/a1) x0 + x1 + (a2/a1) x2  -> fp16 [112,112]
  matmul 1 (PE, fp16)  A1t[w,(u,i)] = a1 * (Xh'^T @ ThT)   -> PSUM -> fp16
  matmul 2 (PE, fp16)  Yu[i,(v,j)]  = A1t[:,u-slice]^T @ TwT -> PSUM f32
  copies (DVE+ACT)     PSUM (bank-packed u-groups) -> staging tile (fp32)
  DMA (sync+ACT HWDGE) staging -> out[b, :, ch0:ch1, :] per u-group

Head optimizations: 6 input DMAs all in ring position 1-2 (per-ring sem
visibility ~9.4/~10.6us after the ~7us Tile preamble), plane-0 emission
interleaves matmul1 halves with the matmul2 groups they unlock (engines
execute their streams in order), plane-0 copies are explicitly pinned
alternating DVE/ACT, and 7 dummy matmuls warm the PE HAM clock-gate to
2.4 GHz during the otherwise-idle input-load window.

The DRAM output tensor is laid out [b, i, c, j] (spatial-row major) rather
than [b, c, i, j]: the SBUF staging tile [i-partition, (c j)] then maps to
DRAM with an (c j)-contiguous run per partition, so every DMA descriptor is
nsel*448 bytes (0.9-3.5 KB) instead of 448 B.  448 B descriptors sit below
the SDMA 512 B line-rate minimum and measured ~15 GB/s/engine (240 GB/s
aggregate); the big-descriptor form reaches the ~300+ GB/s HBM-per-core
practical limit.  The host unshards with a numpy transpose back to
[b, c, i, j].

fp16 keeps |values| < ~1e3 (well in range); measured end-to-end rel err
~4e-4 vs the fp32 reference.

Sharding: pure data parallel, batch 16 -> 2 per core across 8 cores.
"""

import numpy as np

L = 112
SIZE = 8
BS_PER_CORE = 2
N_CORES = 8
SUB_CHANNELS = {0, 1, 2, 3, 4, 5, 8, 9, 16, 24}

RGB2YCBCR = np.asarray(
    [[0.299, 0.587, 0.114],
     [-0.168736, -0.331264, 0.5],
     [0.5, -0.418688, -0.081312]], np.float32)

# per-u: first selected v (selected v's are the contiguous range [V_LO[u], 8))
V_LO = []
M_START = []
_m = 0
for _u in range(SIZE):
    _sel = [_v for _v in range(SIZE) if _u * SIZE + _v not in SUB_CHANNELS]
    assert _sel == list(range(_sel[0], SIZE))
    V_LO.append(_sel[0])
    M_START.append(_m)
    _m += len(_sel)
assert _m == 54


def _build_consts():
    """ThT[h', u*112+i] = alpha(u)/2 * sum_x h[x,u] * Ah[8i+x, h']  (fp16).

    ThT == TwT (H and W are symmetric); a single [112, 896] fp16 matrix
    serves as the moving operand of both matmuls.
    """
    Lo = L * SIZE
    src = np.arange(Lo) * (L - 1) / (Lo - 1)
    i0 = np.minimum(np.floor(src).astype(np.int64), L - 2)
    w = (src - i0).astype(np.float32)
    A = np.zeros((Lo, L), np.float32)
    A[np.arange(Lo), i0] = 1.0 - w
    A[np.arange(Lo), i0 + 1] = w

    xg = np.arange(SIZE) + 0.5
    ug = np.arange(SIZE)
    h = np.cos(np.outer(xg, ug) * np.pi / SIZE).astype(np.float32)
    alpha = np.ones(SIZE, np.float32)
    alpha[0] = 1.0 / np.sqrt(2.0)

    Ab = A.reshape(L, SIZE, L)  # [i, x, h']
    Th = np.einsum('xu,ixh->uih', h, Ab).astype(np.float32)
    Th = Th * (alpha / 2.0)[:, None, None]
    ThT = np.ascontiguousarray(Th.transpose(2, 0, 1).reshape(L, SIZE * L))
    # The same matrix serves as matmul1's rhs (ThT) and matmul2's rhs (TwT);
    # only one copy is loaded.
    return ThT.astype(np.float16)


_CACHE = {}


def _build_program():
    import concourse.bacc as bacc
    import concourse.mybir as mybir
    import concourse.tile as tile

    f32 = mybir.dt.float32
    f16 = mybir.dt.float16
    mult = mybir.AluOpType.mult
    add = mybir.AluOpType.add

    M2 = (127.5 * RGB2YCBCR).astype(np.float32)

    nc = bacc.Bacc(
        "TRN2",
        target_bir_lowering=False,
        debug=False,
        enable_asserts=False,
        num_devices=N_CORES,
        # the kernel never reads partition_id; dropping it removes the
        # per-engine preamble TENSOR_LOADs (~1.1us each) that fetch it.
        enable_partition_id=False,
    )
    x_d = nc.dram_tensor("x", [BS_PER_CORE, 3, L, L], f32, kind="ExternalInput").ap()
    twt_d = nc.dram_tensor("twt", [L, SIZE * L], f16, kind="ExternalInput").ap()
    # [b, i, c, j]: row-major in the spatial row so the per-u DMA gets
    # (c j)-contiguous DRAM runs per partition (host transposes back).
    out_d = nc.dram_tensor(
        "out", [BS_PER_CORE, L, 162, L], f32, kind="ExternalOutput"
    ).ap()

    with tile.TileContext(nc) as tc:
        with tc.tile_pool(name="consts", bufs=1) as cpool, \
             tc.tile_pool(name="xin", bufs=2) as xpool, \
             tc.tile_pool(name="mix", bufs=3) as mpool, \
             tc.tile_pool(name="work", bufs=3) as wpool, \
             tc.tile_pool(name="outb", bufs=16) as opool, \
             tc.tile_pool(name="ps", bufs=2, space="PSUM") as ppool, \
             tc.tile_pool(name="ps2", bufs=3, space="PSUM") as ppool2:
            # b=0 heads the critical path: load its channels as three
            # parallel DMAs (c0/c1 feed the first mix op) split across both
            # HWDGE rings so the first mix starts as early as possible.
            xbs = []
            for b in range(BS_PER_CORE):
                xb = xpool.tile([L, 3, L], f32, name=f"xb{b}", tag="xb")
                xbs.append(xb)
            twt = cpool.tile([L, SIZE * L], f16, name="twt_sb")
            # 6 input DMAs, every one in ring position 1 or 2 (per-ring sem
            # visibility: pos1 ~9.4, pos2 ~10.6).  Input loading runs at
            # only ~180 GB/s aggregate (448-2048B descriptors), so total
            # input bytes (501KB) set the input-phase length.
            nc.sync.dma_start(xbs[0][:, 0, :], x_d[0, 0])
            nc.scalar.dma_start(xbs[0][:, 1, :], x_d[0, 1])
            nc.gpsimd.dma_start(xbs[0][:, 2, :], x_d[0, 2])
            nc.sync.dma_start(twt[:, 0:448], twt_d[:, 0:448])
            nc.scalar.dma_start(twt[:, 448:896], twt_d[:, 448:896])
            nc.gpsimd.dma_start(xbs[1][:], x_d[1].transpose([1, 0, 2]))

            # Balance PSUM->SBUF copies between DVE and ACT by measured ns
            # (cayman-errata: DVE (265+FD)/0.96, ACT (475+FD)/1.2 for f32
            # PSUM reads).  DVE also carries the per-plane mix; ACT also
            # issues half the output DMAs (both charged to the tallies).
            eng_ns = {"v": 0.0, "s": 0.0}

            def psum_copy(dst, src, ncols, scale=None, eng=None):
                v_cost = (265.0 + ncols) / 0.96
                s_cost = (475.0 + ncols) / 1.2
                if eng is None:
                    eng = ("v" if eng_ns["v"] + v_cost <= eng_ns["s"] + s_cost
                           else "s")
                if eng == "v":
                    if scale is None:
                        nc.vector.tensor_copy(dst, src)
                    else:
                        nc.vector.tensor_scalar_mul(dst, src, scale)
                    eng_ns["v"] += v_cost
                else:
                    if scale is None:
                        nc.scalar.copy(dst, src)
                    else:
                        nc.scalar.mul(dst, src, scale)
                    eng_ns["s"] += s_cost

            n_dma = [0]

            def emit_mix(b, r):
                """RGB->normalized-Y'CbCr' plane mix on DVE; returns xh."""
                xb = xbs[b]
                tmp = mpool.tile([L, L], f32, name=f"tmp{b}{r}", tag="tmp")
                xh = mpool.tile([L, L], f16, name=f"xh{b}{r}", tag="xh")
                # Xh' = (a0/a1) x0 + x1 + (a2/a1) x2; a1 re-applied by the
                # a1t scale-on-copy.
                nc.vector.scalar_tensor_tensor(
                    tmp[:], xb[:, 0, :], float(M2[r, 0] / M2[r, 1]),
                    xb[:, 1, :], mult, add)
                nc.vector.scalar_tensor_tensor(
                    xh[:], xb[:, 2, :], float(M2[r, 2] / M2[r, 1]),
                    tmp[:], mult, add)
                eng_ns["v"] += 2 * 330.0
                return xh

            def emit_mm1_slice(b, r, a1t, xh, lo, hi, eng=None):
                """matmul1 cols [lo:hi): A1t[w, (u,i)] = a1*(Xh'^T @ ThT).

                ThT and TwT are the same matrix; matmul1 reuses the twt
                tile as its moving operand."""
                w = hi - lo
                ps = ppool.tile([L, 512], f32, name=f"psA{b}{r}{lo}",
                                tag="ps")
                nc.tensor.matmul(
                    ps[0:L, 0:w],
                    lhsT=xh[:],
                    rhs=twt[:, lo:hi],
                    start=True, stop=True)
                # the mix's 1/M2[r,1] normalization is undone here for
                # free (scale-on-copy).
                psum_copy(a1t[:, lo:hi], ps[0:L, 0:w], w,
                          scale=float(M2[r, 1]), eng=eng)

            def emit_stage1(k, b, r):
                """mix + matmul1 for plane k; returns the a1t tile."""
                xh = emit_mix(b, r)
                a1t = wpool.tile([L, SIZE * L], f16, name=f"a1t{b}{r}",
                                 tag="a1t")
                for half in range(2):
                    emit_mm1_slice(b, r, a1t, xh, 448 * half, 448 * (half + 1))
                return a1t

            # matmul2 u-groups: each group's chunks are packed into one
            # 2-bank PSUM tile at offsets chosen so (a) every matmul output
            # stays inside a single 2KB bank and (b) the used columns are
            # one contiguous run -> ONE PSUM->SBUF copy per group instead
            # of two (the per-op overhead is ~265-475 engine cycles).
            # (us, twt_lo, twt_hi, psum_off) per chunk; m channel range.
            GROUPS = []
            GROUPS.append(([(0, 672, 896, 64), (1, 224, 448, 288),
                            (1, 448, 896, 512)], 0, 8, 64))
            GROUPS.append(([(2, 112, 448, 176), (2, 448, 896, 512)],
                           8, 15, 176))
            GROUPS.append(([(3, 112, 448, 176), (3, 448, 896, 512)],
                           15, 22, 176))
            for _u in range(4, 8):
                GROUPS.append(([(_u, 0, 448, 64), (_u, 448, 896, 512)],
                               22 + (_u - 4) * 8, 30 + (_u - 4) * 8, 64))
            # plane-0 variant: u0 and u1 split so the first (tiny) DMA
            # fires as early as possible.
            GROUPS0 = [([(0, 672, 896, 0)], 0, 2, 0),
                       ([(1, 224, 448, 288), (1, 448, 896, 512)], 2, 8, 288)]
            GROUPS0 += GROUPS[1:]

            def emit_group(b, r, a1t, gi, group, eng=None):
                """matmul2 + staging copy + output DMA for one u-group."""
                chunks, mlo, mhi, off = group
                ncols = (mhi - mlo) * L
                pp = ppool2.tile([128, 1024], f32, name=f"pp{b}{r}{gi}",
                                 tag="pp")
                for us, tlo, thi, poff in chunks:
                    nc.tensor.matmul(
                        pp[0:L, poff:poff + (thi - tlo)],
                        lhsT=a1t[:, us * L:(us + 1) * L],
                        rhs=twt[:, tlo:thi],
                        start=True, stop=True)
                ust = opool.tile([L, ncols], f32,
                                 name=f"ust{b}{r}{gi}", tag="ust")
                psum_copy(ust[:], pp[0:L, off:off + ncols], ncols, eng=eng)

                # dst [i, c, j]: per partition i the (c, j) run is
                # contiguous in DRAM -> one (mhi-mlo)*448B descriptor
                # per partition.  1:1 over the two HWDGE rings (ACT's
                # issue time is charged to its copy balance).
                c0 = r * 54 + mlo
                if n_dma[0] % 2 == 1:
                    nc.scalar.dma_start(out_d[b, :, c0:c0 + (mhi - mlo)],
                                        ust[:])
                    eng_ns["s"] += 700.0
                else:
                    nc.sync.dma_start(out_d[b, :, c0:c0 + (mhi - mlo)],
                                      ust[:])
                n_dma[0] += 1

            def emit_stage2(b, r, a1t):
                for gi, g in enumerate(GROUPS):
                    emit_group(b, r, a1t, gi, g)

            # Plane 0 heads the critical path.  Engines execute their
            # instruction streams IN ORDER, so matmul1 slices are emitted
            # interleaved with exactly the matmul2 groups they unlock: a
            # slice gated on a late tht quarter-load must not precede (in
            # PE program order) groups whose inputs are already resident.
            # HAM warm-up: the PE powers on clock-gated at 1.2 GHz and
            # needs ~3.4us of activity to reach 2.4 GHz.  The PE is idle
            # during the input-load window anyway, so burn it with dummy
            # matmuls on an uninitialized scratch tile (results land in a
            # PSUM bank that every real matmul later overwrites with
            # start=True); the real plane-0 stream then runs at full clock.
            dmy = cpool.tile([L, 512], f16, name="ham_warm")
            nc.vector.memset(dmy[:], 1.0)
            # 6 dummies end ~10.5us, dovetailing with matmul1's twt-load
            # gate; a 7th would push the real PE stream back ~0.4us.
            for hw_i in range(6):
                psd = ppool2.tile([128, 1024], f32, name=f"hw{hw_i}",
                                  tag="pp")
                nc.tensor.matmul(
                    psd[0:L, 0:512],
                    lhsT=dmy[:, 0:112],
                    rhs=dmy[:],
                    start=True, stop=True)

            # Copies in the plane-0 block are explicitly pinned, strictly
            # alternating DVE/ACT in gate order: the greedy balancer would
            # otherwise queue a critical copy behind hoisted later-plane
            # mixes on DVE while ACT sits idle (seen in profile).
            b0, r0 = 0, 0
            xh0 = emit_mix(b0, r0)
            a1t0 = wpool.tile([L, SIZE * L], f16, name="a1t00", tag="a1t")
            emit_mm1_slice(b0, r0, a1t0, xh0, 0, 448, eng="v")   # u0-u3 lhsT
            emit_group(b0, r0, a1t0, 0, GROUPS0[0], eng="s")     # u0
            emit_group(b0, r0, a1t0, 1, GROUPS0[1], eng="v")     # u1
            emit_group(b0, r0, a1t0, 2, GROUPS0[2], eng="s")     # u2
            emit_group(b0, r0, a1t0, 3, GROUPS0[3], eng="v")     # u3
            emit_mm1_slice(b0, r0, a1t0, xh0, 448, 896, eng="s")  # u4-u7 lhsT
            for gi in range(4, 8):
                emit_group(b0, r0, a1t0, gi, GROUPS0[gi],
                           eng=("v" if gi % 2 == 0 else "s"))    # u4-u7

            # Remaining planes, software-pipelined: plane k+1's mix/matmul1
            # is emitted before plane k's matmul2 stream, so the PE never
            # idles across the a1t copy boundary between planes.
            planes = [(b, r) for b in range(BS_PER_CORE) for r in range(3)][1:]
            prev = None
            for k, (b, r) in enumerate(planes):
                a1t = emit_stage1(k, b, r)
                if prev is not None:
                    emit_stage2(prev[0], prev[1], prev[2])
                prev = (b, r, a1t)
            emit_stage2(prev[0], prev[1], prev[2])

    nc.compile()
    return nc


def kernel(x: np.ndarray) -> np.ndarray:
    from concourse import bass_utils

    x = np.ascontiguousarray(np.asarray(x, np.float32))
    assert x.shape == (BS_PER_CORE * N_CORES, 3, L, L)

    if "nc" not in _CACHE:
        _CACHE["nc"] = _build_program()
        _CACHE["consts"] = _build_consts()
    nc = _CACHE["nc"]
    TwT = _CACHE["consts"]

    in_maps = [
        {"x": x[c * BS_PER_CORE:(c + 1) * BS_PER_CORE], "twt": TwT}
        for c in range(N_CORES)
    ]
    res = bass_utils.run_bass_kernel_spmd(nc, in_maps, core_ids=list(range(N_CORES)))
    # [16, 112, 162, 112] (b, i, c, j) -> [16, 162, 112, 112]
    out = np.concatenate([res.results[c]["out"] for c in range(N_CORES)], axis=0)
    return np.ascontiguousarray(out.transpose(0, 2, 1, 3))
